# revision 29
# baseline (speedup 1.0000x reference)
"""ArcMarginProduct (ArcFace) forward on 8 TRN2 NeuronCores.

out[b, c] = s * cos(theta_bc)         except at c == label[b] where
out[b, c] = s * phi(cos(theta_bc))    (margin epilogue)

Strategy (classification-parallel / Partial-FC), v8:
  - pad C 84281 -> 86016 = 8 * 10752, shard class rows across 8 cores
  - host precomputes xt = bf16((s * x / ||x||).T)  [D, B] and
    wnT = bf16((w / ||w_c||).T)  [D, CS] per shard -> the device kernel
    is a pure bf16 matmul: out^T[c, b] = wnT^T @ xt, PE-bound, with no
    on-device transposes, casts, or normalization
  - margin epilogue (512 scattered label positions) applied on host
  - wnT-load DMAs ride the scalar(Act) HW DGE queue (prefetch depth 2),
    out-store DMAs the sync(SP) queue -> independent streams
  - PSUM->SBUF eviction split: scalar takes 2 of 4 class windows
    (activation Copy), vector the other 2 (tensor_copy)
  - host concatenates shards, drops padding, transposes, casts to f32
"""

import math

import numpy as np

B = 512
D = 512
C = 84281
NCORES = 8
W = 83                 # matmul M-windows of 128 classes per core
CS = W * 128           # 10624 padded classes per core
TILES = [(4 * t, 4) for t in range(20)] + [(80, 2), (82, 1)]
NT = len(TILES)        # 22 tiles: 20 x 4 windows + tapered 2 + 1 tail
REAL = [10536] * 7 + [C - 10536 * 7]   # real class rows per core
BASE = [10536 * i for i in range(NCORES)]
PF = 2                 # w-DMA prefetch depth in tiles

S_SCALE = 32.0
MARGIN = 0.5
COS_M = math.cos(MARGIN)
SIN_M = math.sin(MARGIN)
TH = math.cos(math.pi - MARGIN)
MM = math.sin(math.pi - MARGIN) * MARGIN

_CACHE = {}


def _build_nc():
    import concourse.tile as tile
    from concourse import bacc, mybir
    from contextlib import ExitStack

    bf16 = mybir.dt.bfloat16
    f32 = mybir.dt.float32

    nc = bacc.Bacc("TRN2", target_bir_lowering=False, debug=False, num_devices=NCORES)
    w_ext = nc.declare_dram_parameter("wnt", [D, CS], bf16, isOutput=False)
    xt_ext = nc.declare_dram_parameter("xt", [D, B], bf16, isOutput=False)
    out_ext = nc.declare_dram_parameter("out", [CS, B], bf16, isOutput=True)

    # class g = w*128 + p  ->  M-window w, psum partition p
    w_view = w_ext[:].rearrange("(k p) c -> p k c", p=128)          # [128, 4, CS]
    xt_view = xt_ext[:].rearrange("(k p) b -> p k b", p=128)        # [128, 4, B]
    out_view = out_ext[:].rearrange("(w p) b -> p w b", p=128)      # [128, W, B]

    with tile.TileContext(nc) as tc, ExitStack() as es:
        cpool = es.enter_context(tc.tile_pool(name="consts", bufs=1))
        wpool = es.enter_context(tc.tile_pool(name="wch", bufs=4))
        outpool = es.enter_context(tc.tile_pool(name="outch", bufs=3))
        ppool_out = es.enter_context(tc.tile_pool(name="pout", bufs=4, space="PSUM"))

        # ---- w prefetch (scalar HWDGE queue)
        wch_tiles = []

        def issue_w_dma(t):
            w0, nm = TILES[t]
            c0, c1 = w0 * 128, (w0 + nm) * 128
            wch = wpool.tile([128, 4, 512], bf16, tag="wch", name="wch")
            if t == 0:
                # split the first load so the very first matmul (k=0, m=0)
                # can start as soon as its 32KB lands
                nc.scalar.dma_start(
                    out=wch[:, 0, 0:128], in_=w_view[:, 0, c0 : c0 + 128]
                )
                nc.scalar.dma_start(
                    out=wch[:, 1:4, 0:128], in_=w_view[:, 1:4, c0 : c0 + 128]
                )
                nc.scalar.dma_start(
                    out=wch[:, :, 128 : c1 - c0], in_=w_view[:, :, c0 + 128 : c1]
                )
            elif t == 1:
                # startup balance: the sync queue only carries xt (0.5MB)
                # during warmup while scalar has w0+w1 -- route w1 there
                nc.sync.dma_start(
                    out=wch[:, :, 0 : c1 - c0], in_=w_view[:, :, c0:c1]
                )
            else:
                nc.scalar.dma_start(
                    out=wch[:, :, 0 : c1 - c0], in_=w_view[:, :, c0:c1]
                )
            wch_tiles.append(wch)

        # ---- one-shot load: xt (pre-normalized, pre-scaled, bf16),
        # one DMA per k slice so each k-accumulation can begin early
        xnT = cpool.tile([128, 4, B], bf16, tag="xnT")
        for k in range(4):
            nc.sync.dma_start(out=xnT[:, k, :], in_=xt_view[:, k, :])

        for t in range(PF):
            issue_w_dma(t)

        def pe(t):
            wch = wch_tiles[t]
            if t + PF < NT:
                issue_w_dma(t + PF)
            nm = TILES[t][1]
            pos = [
                ppool_out.tile([128, 2 * B], f32, name="po")
                for _ in range((nm + 1) // 2)
            ]
            # k-chains interleaved across PSUM banks: consecutive matmuls
            # hit different banks so stop->start turnaround overlaps
            for k in range(4):
                for m in range(nm):
                    po = pos[m // 2]
                    nc.tensor.matmul(
                        po[:, (m % 2) * B : (m % 2 + 1) * B],
                        lhsT=wch[:, k, m * 128 : (m + 1) * 128],
                        rhs=xnT[:, k, :],
                        start=(k == 0),
                        stop=(k == 3),
                    )
            return pos

        def outcopy(t, pos, split_store=False):
            """PSUM -> SBUF eviction (scalar m0/m1, vector m2+), SP store."""
            w0, nm = TILES[t]
            outch = outpool.tile([128, 4, B], bf16, tag="outch", name="outch")
            for m in range(nm):
                src = pos[m // 2][:, (m % 2) * B : (m % 2 + 1) * B]
                if m % 2 == 0:
                    nc.scalar.activation(
                        out=outch[:, m, :],
                        in_=src,
                        func=mybir.ActivationFunctionType.Copy,
                    )
                else:
                    nc.vector.tensor_copy(outch[:, m, :], src)
            if split_store and nm >= 2:
                half = nm // 2
                nc.sync.dma_start(
                    out=out_view[:, w0 : w0 + half, :], in_=outch[:, 0:half, :]
                )
                nc.sync.dma_start(
                    out=out_view[:, w0 + half : w0 + nm, :],
                    in_=outch[:, half:nm, :],
                )
            else:
                nc.sync.dma_start(
                    out=out_view[:, w0 : w0 + nm, :], in_=outch[:, 0:nm, :]
                )

        pos_prev = None
        for t in range(NT):
            if pos_prev is not None:
                outcopy(t - 1, pos_prev)
            pos = pe(t)
            pos_prev = pos
        outcopy(NT - 1, pos_prev, split_store=True)

    nc.finalize()
    return nc


def _get_nc():
    if "nc" not in _CACHE:
        _CACHE["nc"] = _build_nc()
    return _CACHE["nc"]


def make_in_maps(x, weight, label):
    import ml_dtypes

    x = np.asarray(x, dtype=np.float32)
    weight = np.asarray(weight, dtype=np.float32)
    xn = x / np.maximum(np.linalg.norm(x, axis=1, keepdims=True), 1e-12)
    xt = np.ascontiguousarray((S_SCALE * xn).T).astype(ml_dtypes.bfloat16)
    wn = weight / np.maximum(
        np.sqrt(np.einsum("cd,cd->c", weight, weight))[:, None], 1e-12
    )
    in_maps = []
    for i in range(NCORES):
        a, r = BASE[i], REAL[i]
        wshard = np.empty((CS, D), dtype=np.float32)
        wshard[:r] = wn[a : a + r]
        wshard[r:] = 1.0
        wnt = np.ascontiguousarray(wshard.T).astype(ml_dtypes.bfloat16)
        in_maps.append({"wnt": wnt, "xt": xt})
    return in_maps


def assemble(results, label):
    shards = [np.asarray(results[i]["out"])[: REAL[i]] for i in range(NCORES)]
    full_t = np.concatenate(shards, axis=0).astype(np.float32)  # [C, B]
    out = np.ascontiguousarray(full_t.T)                        # [B, C]
    # margin epilogue on the 512 label positions
    label = np.asarray(label).astype(np.int64)
    b = np.arange(B)
    cosv = out[b, label] / S_SCALE
    sine = np.sqrt(np.maximum(0.0, 1.0 - cosv * cosv))
    phi = cosv * COS_M - sine * SIN_M
    out[b, label] = np.where(cosv - TH > 0, phi, cosv - MM) * S_SCALE
    return out


def kernel(x, weight, label):
    from concourse.bass_utils import run_bass_kernel_spmd

    nc = _get_nc()
    in_maps = make_in_maps(x, weight, label)
    res = run_bass_kernel_spmd(nc, in_maps, list(range(NCORES)))
    return assemble(res.results, label)


# revision 30
# speedup vs baseline: 1.0063x; 1.0063x over previous
"""ArcMarginProduct (ArcFace) forward on 8 TRN2 NeuronCores.

out[b, c] = s * cos(theta_bc)         except at c == label[b] where
out[b, c] = s * phi(cos(theta_bc))    (margin epilogue)

Strategy (classification-parallel / Partial-FC), v8:
  - pad C 84281 -> 86016 = 8 * 10752, shard class rows across 8 cores
  - host precomputes xt = bf16((s * x / ||x||).T)  [D, B] and
    wnT = bf16((w / ||w_c||).T)  [D, CS] per shard -> the device kernel
    is a pure bf16 matmul: out^T[c, b] = wnT^T @ xt, PE-bound, with no
    on-device transposes, casts, or normalization
  - margin epilogue (512 scattered label positions) applied on host
  - wnT-load DMAs ride the scalar(Act) HW DGE queue (prefetch depth 2),
    out-store DMAs the sync(SP) queue -> independent streams
  - PSUM->SBUF eviction split: scalar takes 2 of 4 class windows
    (activation Copy), vector the other 2 (tensor_copy)
  - host concatenates shards, drops padding, transposes, casts to f32
"""

import math

import numpy as np

B = 512
D = 512
C = 84281
NCORES = 8
W = 83                 # matmul M-windows of 128 classes per core
CS = W * 128           # 10624 padded classes per core
TILES = [(4 * t, 4) for t in range(20)] + [(80, 2), (82, 1)]
NT = len(TILES)        # 22 tiles: 20 x 4 windows + tapered 2 + 1 tail
REAL = [10536] * 7 + [C - 10536 * 7]   # real class rows per core
BASE = [10536 * i for i in range(NCORES)]
PF = 2                 # w-DMA prefetch depth in tiles

S_SCALE = 32.0
MARGIN = 0.5
COS_M = math.cos(MARGIN)
SIN_M = math.sin(MARGIN)
TH = math.cos(math.pi - MARGIN)
MM = math.sin(math.pi - MARGIN) * MARGIN

_CACHE = {}


def _build_nc():
    import concourse.tile as tile
    from concourse import bacc, mybir
    from contextlib import ExitStack

    bf16 = mybir.dt.bfloat16
    f32 = mybir.dt.float32

    nc = bacc.Bacc("TRN2", target_bir_lowering=False, debug=False, num_devices=NCORES)
    w_ext = nc.declare_dram_parameter("wnt", [D, CS], bf16, isOutput=False)
    xt_ext = nc.declare_dram_parameter("xt", [D, B], bf16, isOutput=False)
    out_ext = nc.declare_dram_parameter("out", [CS, B], bf16, isOutput=True)

    # class g = w*128 + p  ->  M-window w, psum partition p
    w_view = w_ext[:].rearrange("(k p) c -> p k c", p=128)          # [128, 4, CS]
    xt_view = xt_ext[:].rearrange("(k p) b -> p k b", p=128)        # [128, 4, B]
    out_view = out_ext[:].rearrange("(w p) b -> p w b", p=128)      # [128, W, B]

    with tile.TileContext(nc) as tc, ExitStack() as es:
        cpool = es.enter_context(tc.tile_pool(name="consts", bufs=1))
        wpool = es.enter_context(tc.tile_pool(name="wch", bufs=4))
        outpool = es.enter_context(tc.tile_pool(name="outch", bufs=3))
        ppool_out = es.enter_context(tc.tile_pool(name="pout", bufs=4, space="PSUM"))

        # ---- w prefetch (scalar HWDGE queue)
        wch_tiles = []

        def issue_w_dma(t):
            w0, nm = TILES[t]
            c0, c1 = w0 * 128, (w0 + nm) * 128
            wch = wpool.tile([128, 4, 512], bf16, tag="wch", name="wch")
            if t == 0:
                # split the first load so the very first matmul (k=0, m=0)
                # can start as soon as its 32KB lands
                nc.scalar.dma_start(
                    out=wch[:, 0, 0:128], in_=w_view[:, 0, c0 : c0 + 128]
                )
                nc.scalar.dma_start(
                    out=wch[:, 1:4, 0:128], in_=w_view[:, 1:4, c0 : c0 + 128]
                )
                nc.scalar.dma_start(
                    out=wch[:, :, 128 : c1 - c0], in_=w_view[:, :, c0 + 128 : c1]
                )
            elif t == 1:
                # startup balance: the sync queue only carries xt (0.5MB)
                # during warmup while scalar has w0+w1 -- route w1 there
                nc.sync.dma_start(
                    out=wch[:, :, 0 : c1 - c0], in_=w_view[:, :, c0:c1]
                )
            else:
                nc.scalar.dma_start(
                    out=wch[:, :, 0 : c1 - c0], in_=w_view[:, :, c0:c1]
                )
            wch_tiles.append(wch)

        # ---- one-shot load: xt (pre-normalized, pre-scaled, bf16),
        # one DMA per k slice so each k-accumulation can begin early
        xnT = cpool.tile([128, 4, B], bf16, tag="xnT")
        for k in range(4):
            nc.sync.dma_start(out=xnT[:, k, :], in_=xt_view[:, k, :])

        for t in range(PF):
            issue_w_dma(t)

        def pe(t):
            wch = wch_tiles[t]
            if t + PF < NT:
                issue_w_dma(t + PF)
            nm = TILES[t][1]
            pos = []
            for g0 in (0, 2):
                if g0 >= nm:
                    break
                po = ppool_out.tile([128, 2 * B], f32, name="po")
                for jj in range(min(2, nm - g0)):
                    m = g0 + jj
                    for k in range(4):
                        nc.tensor.matmul(
                            po[:, jj * B : (jj + 1) * B],
                            lhsT=wch[:, k, m * 128 : (m + 1) * 128],
                            rhs=xnT[:, k, :],
                            start=(k == 0),
                            stop=(k == 3),
                        )
                pos.append(po)
            return pos

        def outcopy(t, pos, split_store=False):
            """PSUM -> SBUF eviction (scalar m0/m1, vector m2+), SP store."""
            w0, nm = TILES[t]
            outch = outpool.tile([128, 4, B], bf16, tag="outch", name="outch")
            for m in range(nm):
                src = pos[m // 2][:, (m % 2) * B : (m % 2 + 1) * B]
                if m % 2 == 0:
                    nc.scalar.activation(
                        out=outch[:, m, :],
                        in_=src,
                        func=mybir.ActivationFunctionType.Copy,
                    )
                else:
                    nc.vector.tensor_copy(outch[:, m, :], src)
            if split_store and nm >= 2:
                half = nm // 2
                nc.sync.dma_start(
                    out=out_view[:, w0 : w0 + half, :], in_=outch[:, 0:half, :]
                )
                nc.sync.dma_start(
                    out=out_view[:, w0 + half : w0 + nm, :],
                    in_=outch[:, half:nm, :],
                )
            else:
                nc.sync.dma_start(
                    out=out_view[:, w0 : w0 + nm, :], in_=outch[:, 0:nm, :]
                )

        pos_prev = None
        for t in range(NT):
            if pos_prev is not None:
                outcopy(t - 1, pos_prev)
            pos = pe(t)
            pos_prev = pos
        outcopy(NT - 1, pos_prev, split_store=True)

    nc.finalize()
    return nc


def _get_nc():
    if "nc" not in _CACHE:
        _CACHE["nc"] = _build_nc()
    return _CACHE["nc"]


def make_in_maps(x, weight, label):
    import ml_dtypes

    x = np.asarray(x, dtype=np.float32)
    weight = np.asarray(weight, dtype=np.float32)
    xn = x / np.maximum(np.linalg.norm(x, axis=1, keepdims=True), 1e-12)
    xt = np.ascontiguousarray((S_SCALE * xn).T).astype(ml_dtypes.bfloat16)
    wn = weight / np.maximum(
        np.sqrt(np.einsum("cd,cd->c", weight, weight))[:, None], 1e-12
    )
    in_maps = []
    for i in range(NCORES):
        a, r = BASE[i], REAL[i]
        wshard = np.empty((CS, D), dtype=np.float32)
        wshard[:r] = wn[a : a + r]
        wshard[r:] = 1.0
        wnt = np.ascontiguousarray(wshard.T).astype(ml_dtypes.bfloat16)
        in_maps.append({"wnt": wnt, "xt": xt})
    return in_maps


def assemble(results, label):
    shards = [np.asarray(results[i]["out"])[: REAL[i]] for i in range(NCORES)]
    full_t = np.concatenate(shards, axis=0).astype(np.float32)  # [C, B]
    out = np.ascontiguousarray(full_t.T)                        # [B, C]
    # margin epilogue on the 512 label positions
    label = np.asarray(label).astype(np.int64)
    b = np.arange(B)
    cosv = out[b, label] / S_SCALE
    sine = np.sqrt(np.maximum(0.0, 1.0 - cosv * cosv))
    phi = cosv * COS_M - sine * SIN_M
    out[b, label] = np.where(cosv - TH > 0, phi, cosv - MM) * S_SCALE
    return out


def kernel(x, weight, label):
    from concourse.bass_utils import run_bass_kernel_spmd

    nc = _get_nc()
    in_maps = make_in_maps(x, weight, label)
    res = run_bass_kernel_spmd(nc, in_maps, list(range(NCORES)))
    return assemble(res.results, label)
